# revision 30
# baseline (speedup 1.0000x reference)
# DiabaticReadout forward on Trainium2 (Bass/Tile), 8-core data-parallel.
#
# Per sample i: H = [[d0, lam], [lam, d1]] (2x2 symmetric).  Eigenvalues in
# closed form:
#   mean    = 0.5*(d0+d1)
#   halfgap = sqrt(0.25*((d0-d1)^2 + 4*lam^2))
#   e0, e1  = mean -/+ halfgap          (ascending, matches eigh)
#
# Purely elementwise -> shard the N axis across the 8 NeuronCores, each core
# streams [128, F] tiles.  The kernel is HBM-DMA-bound, so all device I/O is
# fp16: inputs are downcast on the host (rel err ~9e-4 vs the 2e-2 gate),
# outputs come back fp16 and the host upcasts during the unshard.  The host
# prescales d0,d1 by 0.5 (exact: exponent decrement, fused into the fp16
# convert), which makes every device-side scale factor vanish:
#   mean = d0' + d1'          r = sqrt((d0'-d1')^2 + lam^2)
#   e0, e1 = mean -/+ r
# so the whole DVE side is plain tensor_tensor ops, which (unlike
# scalar_tensor_tensor) run in the packed 2x mode for unit-stride fp16.
# ACT does the Squares and the Sqrt (1 elem/cycle/lane regardless of
# dtype); lam^2 alternates between ACT and DVE per tile (l2_engine="alt")
# to balance the two engines' busy time under the DMA stream.
#
# DMA structure: the host packs the three inputs into ONE per-core [3, C]
# tensor and the outputs come back as ONE [2, C] tensor, so each tile is a
# single load DMA ([128, 3, F] - three runs per partition) and a single
# store DMA ([128, 2, F]), minimizing trigger + semaphore overhead.  Loads
# are issued from the SP HWDGE ring and stores from the ACT HWDGE ring:
# separate rings so stores don't queue FIFO behind all the loads, and
# HWDGE for both because SWDGE (gpsimd) descriptor generation gets locked
# out of the shared SBUF port pair while DVE runs 2x perf-mode ops.
# A small start ramp (256/512/1024-row tiles) gets compute going ~4us
# earlier than a full 2048-row first tile would.
#
# Measured on this fixture: per-core HBM runs ~417 GB/s when active, so the
# 12.5 MB/core of fp16 traffic is a ~30 us streaming floor; NEFF boot
# (engine barrier + instruction upload) is ~9 us and the exit barrier +
# final store receipt ~3 us.  DVE work is 5 passes (28 us) and ACT 3
# passes (28 us), both just under the DMA time, which lands the kernel at
# ~53 us median (the fixture drifts between a ~52.5 us fast mode and a
# ~57 us slow mode run-to-run).

import numpy as np

import concourse.bacc as bacc_mod
import concourse.tile as tile
from concourse import bacc, mybir
from concourse.bass_utils import run_bass_kernel_spmd

import contextlib


@contextlib.contextmanager
def _pin_act_table(keep="sqrt_and_others"):
    """All our activations (Square, Sqrt, Copy) live in the single
    `sqrt_and_others` set, but the table-load pass greedily picks the first
    set containing each function, which alternates tables per tile
    (~2.5us/tile of ACT_TABLE_LOAD thrash).  Present every other set as
    empty during compile so the pass pins everything to one table; indices
    stay aligned with act_info.json."""
    orig = bacc_mod.get_activation_tables

    def patched(arch):
        t = orig(arch)
        assert keep in t, sorted(t)
        return {name: (funcs if name == keep else set()) for name, funcs in t.items()}

    bacc_mod.get_activation_tables = patched
    try:
        yield
    finally:
        bacc_mod.get_activation_tables = orig

N_CORES = 8
P = 128  # SBUF partitions
_USE_PE_DEFAULT = False

_cache = {}


def _tile_schedule(rows, f_tile, ramp, ramp_end=()):
    """Tile-size schedule: optional small prologue/epilogue tiles so the
    pipeline fills/drains quickly, f_tile-sized tiles in the middle."""
    head, tail = [], []
    left = rows
    for s in ramp:
        if left <= 0:
            break
        s = min(s, left)
        head.append(s)
        left -= s
    for s in ramp_end:
        if left <= 0:
            break
        s = min(s, left)
        tail.append(s)
        left -= s
    mid = []
    while left > 0:
        s = min(f_tile, left)
        mid.append(s)
        left -= s
    return head + mid + tail[::-1]


def _build(rows, f_tile=2048, in_bufs=4, out_bufs=5, tmp_bufs=3,
           sum_bufs=6, l2_bufs=4,
           sum_engine="vector", store_engine="scalar", e1_engine="vector",
           load_engine="sync", alias_tmps=False, dif_first=True,
           d2_engine="scalar", l2_engine="alt", pure_copy=False,
           use_pe=False, ramp=(256, 512, 1024), ramp_end=(512,)):
    """Per-core Bass module: input din [3, P*rows] fp16 (d0', d1', lam),
    output eout [2, P*rows] fp16 (e0, e1)."""
    C = P * rows
    f16 = mybir.dt.float16
    Act = mybir.ActivationFunctionType

    nc = bacc.Bacc(
        "TRN2",
        target_bir_lowering=False,
        debug=False,
        num_devices=N_CORES,
    )
    din = nc.dram_tensor("din", [3 * C], f16, kind="ExternalInput").ap()
    eout = nc.dram_tensor("eout", [2 * C], f16, kind="ExternalOutput").ap()
    ident = None
    if use_pe:
        ident = nc.dram_tensor("ident", [P, P], f16, kind="ExternalInput").ap()

    dinv = din.rearrange("(t p f) -> p t f", t=3, p=P)
    eoutv = eout.rearrange("(t p f) -> p t f", t=2, p=P)

    sum_eng = getattr(nc, sum_engine)
    store_eng = getattr(nc, store_engine)
    e1_eng = getattr(nc, e1_engine)
    load_eng = getattr(nc, load_engine)
    sizes = _tile_schedule(rows, f_tile, ramp, ramp_end)

    def sq(eng_name, out, in_, i=0):
        if eng_name == "alt":  # balance ACT vs DVE load across tiles
            eng_name = "vector" if i % 2 else "scalar"
        elif eng_name == "alt2":  # opposite phase
            eng_name = "scalar" if i % 2 else "vector"
        if eng_name == "scalar":
            nc.scalar.activation(out, in_, Act.Square)
        else:
            getattr(nc, eng_name).tensor_mul(out, in_, in_)

    # Software-pipelined emission: each engine's (in-order) instruction
    # stream only contains ops whose cross-engine producers ran >=1 stage
    # (one whole tile) earlier, so sequencers never block mid-stream.
    # Stages (tile k at iteration i):
    #   S0 i=k   : load(k)
    #   S1 i=k+1 : dif(k), sum(k) [V]; l2(k) [S or V]
    #   S2 i=k+2 : d2(k) [S]
    #   S3 i=k+3 : s(k) [V]
    #   S4 i=k+4 : sqrt(k) [S]
    #   S5 i=k+5 : e0(k), e1(k) [V]; store(k)
    T = len(sizes)
    offs = np.concatenate([[0], np.cumsum(sizes)]).astype(int)
    st = {}  # per-tile tile handles

    with tile.TileContext(nc) as tc:
        with (
            tc.tile_pool(name="ins", bufs=in_bufs) as ins,
            tc.tile_pool(name="outs", bufs=out_bufs) as outs,
            tc.tile_pool(name="tmp", bufs=tmp_bufs) as tmp,
            tc.tile_pool(name="psum", bufs=2, space="PSUM") as psum,
        ):
            t_ident = None
            if use_pe:
                t_ident = tmp.tile([P, P], f16, tag="ident", bufs=1, name="t_ident")
                load_eng.dma_start(t_ident[:], ident[:, :])
            depth = 1 if pure_copy else 5
            for i in range(T + depth):
                k = i
                if k < T:  # S0: load
                    F = sizes[k]
                    sl = slice(offs[k], offs[k] + F)
                    t_in = ins.tile([P, 3, F], f16, tag="in", name="t_in")
                    load_eng.dma_start(t_in[:], dinv[:, :, sl])
                    st[k] = {"in": t_in, "F": F, "sl": sl}

                if pure_copy:
                    k = i - 1
                    if 0 <= k < T:
                        c = st[k]
                        store_eng.dma_start(
                            eoutv[:, :, c["sl"]], c["in"][:, 0:2, :]
                        )
                        del st[k]
                    continue

                k = i - 1
                if 0 <= k < T:  # S1
                    c = st[k]
                    t_in, F = c["in"], c["F"]
                    v_d0, v_d1 = t_in[:, 0, :], t_in[:, 1, :]
                    c["sum"] = tmp.tile([P, F], f16, tag="sum", bufs=sum_bufs, name="t_sum")
                    c["dif"] = tmp.tile([P, F], f16, tag="dif", name="t_dif")
                    if dif_first:
                        nc.vector.tensor_sub(c["dif"][:], v_d0, v_d1)
                        sum_eng.tensor_add(c["sum"][:], v_d0, v_d1)
                    else:
                        sum_eng.tensor_add(c["sum"][:], v_d0, v_d1)
                        nc.vector.tensor_sub(c["dif"][:], v_d0, v_d1)
                    c["l2"] = tmp.tile([P, F], f16, tag="l2", bufs=l2_bufs, name="t_l2")
                    sq(l2_engine, c["l2"][:], t_in[:, 2, :], k)

                k = i - 2
                if 0 <= k < T:  # S2
                    c = st[k]
                    F = c["F"]
                    c["d2"] = tmp.tile([P, F], f16, tag="dif" if alias_tmps else "d2", name="t_d2")
                    sq(d2_engine, c["d2"][:], c["dif"][:], k)

                k = i - 3
                if 0 <= k < T:  # S3: s = d2 + l2
                    c = st[k]
                    F = c["F"]
                    if use_pe:
                        # Idle Tensor engine: identity-matmul copy-accumulate
                        # into PSUM, 512-col chunks (moving-dim limit).
                        c["s"] = psum.tile([P, F], mybir.dt.float32, tag="s",
                                           name="t_s")
                        for q0 in range(0, F, 512):
                            q1 = min(q0 + 512, F)
                            nc.tensor.matmul(
                                c["s"][:, q0:q1], t_ident[:], c["d2"][:, q0:q1],
                                start=True, stop=False,
                            )
                            nc.tensor.matmul(
                                c["s"][:, q0:q1], t_ident[:], c["l2"][:, q0:q1],
                                start=False, stop=True,
                            )
                    else:
                        c["s"] = tmp.tile([P, F], f16, tag="l2" if alias_tmps else "s", name="t_s")
                        nc.vector.tensor_add(c["s"][:], c["d2"][:], c["l2"][:])

                k = i - 4
                if 0 <= k < T:  # S4
                    c = st[k]
                    F = c["F"]
                    c["r"] = tmp.tile([P, F], f16, tag="dif" if alias_tmps else "r", name="t_r")
                    nc.scalar.activation(c["r"][:], c["s"][:], Act.Sqrt)

                k = i - 5
                if 0 <= k < T:  # S5
                    c = st[k]
                    F = c["F"]
                    t_out = outs.tile([P, 2, F], f16, tag="out", name="t_out")
                    nc.vector.tensor_sub(t_out[:, 0, :], c["sum"][:], c["r"][:])
                    e1_eng.tensor_add(t_out[:, 1, :], c["sum"][:], c["r"][:])
                    store_eng.dma_start(eoutv[:, :, c["sl"]], t_out[:])
                    del st[k]
    with _pin_act_table():
        nc.compile()
    return nc


def _get_nc(rows, **cfg):
    for k in ("ramp", "ramp_end"):
        if k in cfg:
            cfg[k] = tuple(cfg[k])
    key = (rows, tuple(sorted(cfg.items())))
    if key not in _cache:
        _cache[key] = _build(rows, **cfg)
    return _cache[key]


def kernel(d0, d1, lam, _trace=False, **cfg):
    # 0.5 prescale on d0/d1 removes every scale factor device-side (see top).
    d0 = (np.asarray(d0) * 0.5).astype(np.float16).ravel()
    d1 = (np.asarray(d1) * 0.5).astype(np.float16).ravel()
    lam = np.asarray(lam).astype(np.float16).ravel()
    n = d0.shape[0]

    # Per-core sample count: multiple of 128, cores cover ceil(n / 8).
    rows = -(-n // (N_CORES * P))  # ceil
    C = P * rows
    total = N_CORES * C
    pad = total - n
    if pad:
        z = np.zeros(pad, np.float16)
        d0 = np.concatenate([d0, z])
        d1 = np.concatenate([d1, z])
        lam = np.concatenate([lam, z])

    use_pe = cfg.get("use_pe", _USE_PE_DEFAULT)
    cfg.setdefault("use_pe", use_pe)
    eye = np.eye(P, dtype=np.float16) if use_pe else None
    in_maps = []
    for c in range(N_CORES):
        din = np.empty(3 * C, np.float16)
        din[0:C] = d0[c * C : (c + 1) * C]
        din[C : 2 * C] = d1[c * C : (c + 1) * C]
        din[2 * C : 3 * C] = lam[c * C : (c + 1) * C]
        m = {"din": din}
        if use_pe:
            m["ident"] = eye
        in_maps.append(m)

    nc = _get_nc(rows, **cfg)
    res = run_bass_kernel_spmd(
        nc, in_maps, core_ids=list(range(N_CORES)), trace=_trace
    )
    global last_results
    last_results = res
    full = np.empty((total, 2), np.float32)
    for c in range(N_CORES):
        eo = res.results[c]["eout"]
        full[c * C : (c + 1) * C, 0] = eo[0:C]
        full[c * C : (c + 1) * C, 1] = eo[C : 2 * C]
    return full[:n]


last_results = None
